# revision 18
# baseline (speedup 1.0000x reference)
"""Causal dot-product attention on 8 Trainium2 NeuronCores (Bass/Tile).

Shapes: Q,K,V [4,16,2048,64] fp32, mask [2048,2048] bool (tril/causal in
practice; general-mask and no-mask fallback paths included). Output
[4,16,2048,64] fp32.

Sharding: the 64 (batch, head) pairs split 8 per core; each core runs full
attention for its heads. kernel() takes full inputs and returns the full
output; sharding/unsharding happens on the host.

Per-head device algorithm (scores kept TRANSPOSED so no large on-device
transposes are needed):
  - Host pre-transposes Q,K per head to [D, S] and duplicates rows into
    [2D, S] so consecutive matmuls can alternate PE row groups 0-63/64-127
    (LDWEIGHTS for row group A overlaps the in-flight matmul on group B).
    V is stored as [P, NKC, D+1] with a ones column appended (denominator).
  - For each 128-key chunk kc (causal: only q >= 128*kc):
      scoresT strip [128 keys, q cols] = matmul(lhsT=KT_chunk, rhs=QT) into
      PSUM; P = exp(scale*scores) from PSUM (ScalarE spline exp, or for a
      bounded fraction of columns a Schraudolph-style DVE affine-int16 trick
      that materializes exp directly in fp16 bits); the causal diagonal
      chunk is masked AFTER exp by a multiplicative 0/1 triangle.
  - PV: psum OT[d & ones-row, q] += matmul(lhsT=V1_chunk, rhs=PT), so row D
    accumulates the softmax denominator.
  - The (head, kc) loop is FLAT and software-pipelined across heads: PV for
    strip kc is emitted pv_lag strips later, including across the head
    boundary, so the in-order PE queue always has ready work while ScalarE
    runs exp and the next head's QK proceeds immediately.
  - Drain: one DVE copy [D+1, S] PSUM->SBUF fp16 (denominator row rides
    along for free: DVE cost scales with the free dim only), DMA'd out from
    the GpSimd queue so the ScalarE sequencer never issues DMAs.
  - Host divides rows 0..D-1 by row D and transposes back to [S, D].

Matmul operands are cast to fp16 on the host: 1 PE cycle/column and half
the DMA bytes. End-to-end rel error vs the fp32 reference: ~4.8e-4 with
offload=0, ~5.1e-3 at the default offload (DVE Schraudolph exp on ~45% of
columns), both well under the 2e-2 gate. Measured ~174us/exec on hardware
(8 cores SPMD; 213-223us baseline).
"""

import numpy as np

import concourse.bass as bass
import concourse.tile as tile
from concourse import bacc, mybir
from concourse.bass_utils import run_bass_kernel_spmd

B, H, S, D = 4, 16, 2048, 64
NCORES = 8
HEADS = B * H
HPC = HEADS // NCORES  # heads per core
P = 128
NKC = S // P  # key chunks per head
D1 = D + 1
PIECE = 512  # psum bank / fp32 moving-operand limit
SCALE = 1.0 / np.sqrt(D)

F32 = mybir.dt.float32
F16 = mybir.dt.float16
I16 = mybir.dt.int16
EXP = mybir.ActivationFunctionType.Exp

# Schraudolph fp16 exp: bits(int16) = round(A*x + B) => fp16 value ~ exp(x).
SCHR_A = 2.0**10 / np.log(2.0)
SCHR_B = 15300.0  # calibrated for min RMS rel err on N(0,1) inputs

MM_DTYPES = {
    "f32r": mybir.dt.float32r,
    "f16": mybir.dt.float16,
    "f32": mybir.dt.float32,
}
MM_NP = {"f32r": np.float32, "f16": np.float16, "f32": np.float32}


def pieces(a0, a1, w=PIECE):
    # [a0, a1) split at w boundaries (psum bank edges)
    out, a = [], a0
    while a < a1:
        b = min((a // w + 1) * w, a1)
        out.append((a, b))
        a = b
    return out


def build_nc2(
    mm: str = "f16",
    st_w: int = 1024,
    pv_chunks: int = 2,
    pt_bufs: int = 28,
    qk_bufs: int = 3,
    offload: float = 1.0,
    schr_b: float = SCHR_B,
    loop_reps: int = 0,
    do_exp: bool = True,
    do_pv: bool = True,
    dma_once: bool = False,
    mm_w: int = PIECE,
    pv_pos: int = 1,
    dve_scale: float = 1.0,
    pv_same_w: bool = False,
    drain_eng: str = "dve",
    drain_delay: int = 0,
    tri_eng: str = "dve",
    diag_dve: bool = False,
):
    """Causal-only pipelined builder (see module docstring)."""
    DT = MM_DTYPES[mm]
    nc = bacc.Bacc(None, target_bir_lowering=False)

    QT = nc.declare_dram_parameter("QT", [HPC, 2 * D, S], DT, isOutput=False)
    KT = nc.declare_dram_parameter("KT", [HPC, 2 * D, S], DT, isOutput=False)
    V1 = nc.declare_dram_parameter("V1", [HPC, P, NKC, D1], DT, isOutput=False)
    TRI = nc.declare_dram_parameter("TRI", [P, P], DT, isOutput=False)
    OTD = nc.declare_dram_parameter("OTD", [HPC, D1, S], F16, isOutput=True)

    total_cols = HPC * sum(S - P * kc for kc in range(NKC))
    st_bufs = (3 * 1024) // st_w  # 6 banks of PSUM for score strips
    SH = S // 2  # PV accumulates in column halves -> pso is 2 banks

    with tile.TileContext(nc) as tc:
        with (
            tc.tile_pool(name="const", bufs=1) as const,
            tc.tile_pool(name="qk", bufs=qk_bufs) as qk,
            tc.tile_pool(name="vp", bufs=qk_bufs) as vp,
            tc.tile_pool(name="ptp", bufs=pt_bufs) as ptp,
            tc.tile_pool(name="outp", bufs=2) as outp,
            tc.tile_pool(name="pst", bufs=st_bufs, space="PSUM") as pst,
            tc.tile_pool(name="pso", bufs=1, space="PSUM") as pso,
        ):
            tri_t = const.tile([P, P], DT)
            nc.sync.dma_start(out=tri_t, in_=TRI[:])

            import contextlib

            loop_ctx = (
                tc.For_i(0, loop_reps, 1) if loop_reps else contextlib.nullcontext()
            )
            with loop_ctx:
                # static engine-balance bookkeeping (model ns)
                bal = {"act": 0.0, "dve": 0.0, "off": 0}
                state = {}  # per-head tiles
                mm_i = 0

                pvq = []  # deferred PV/drain chunks (closures)
                drainq = []  # (ready_idx, fn) — drains emitted a few strips late

                def pop_chunk(idx):
                    c = pvq.pop(0)
                    if isinstance(c, tuple):
                        drainq.append((idx + drain_delay, c[1]))
                    else:
                        c()

                def enq_half(h, half):
                    # half 0: columns [0, SH), key chunks 0..SH/P-1
                    # half 1: columns [SH, S), all key chunks
                    c0, c1 = (0, SH) if half == 0 else (SH, S)
                    kmax = (c1 - 1) // P

                    def mk_pv(kc):
                        def f():
                            st_ = state[h]
                            ot_ps = st_.get("ot")
                            if ot_ps is None:
                                ot_ps = pso.tile([D1, SH], F32, tag="ot")
                                st_["ot"] = ot_ps
                            q0 = P * kc
                            for a, b in pieces(max(q0, c0), c1, mm_w):
                                last_kc = min(kmax, (b - 1) // P)
                                nc.tensor.matmul(
                                    ot_ps[:, a - c0 : b - c0],
                                    lhsT=st_["v1"][:, 0 if pv_same_w else kc, :],
                                    rhs=st_["pt"][kc][:, a:b],
                                    start=(kc == 0),
                                    stop=(kc == last_kc),
                                )

                        return f

                    def drain():
                        st_ = state[h]
                        ot_ps = st_.pop("ot")
                        if half == 0:
                            ot_sb = outp.tile([D1, S], F16, tag="ot_sb")
                            st_["ot_sb"] = ot_sb
                        if drain_eng == "gpsimd":
                            nc.gpsimd.tensor_copy(st_["ot_sb"][:, c0:c1], ot_ps)
                        elif drain_eng == "act":
                            nc.scalar.copy(st_["ot_sb"][:, c0:c1], ot_ps)
                            bal["act"] += (SH + 222) * 0.8333 + 57
                        else:
                            nc.vector.tensor_copy(st_["ot_sb"][:, c0:c1], ot_ps)
                            bal["dve"] += (SH + 120) * 1.0417 + 100
                        if half == 1:
                            nc.gpsimd.dma_start(out=OTD[h], in_=st_["ot_sb"])
                            del state[h]

                    if do_pv:
                        pvq.extend(mk_pv(kc) for kc in range(kmax + 1))
                        if drain_delay:
                            pvq.append(("drain", drain))
                        else:
                            pvq.append(drain)

                for idx in range(HPC * NKC):
                    h, kc = divmod(idx, NKC)
                    if kc == 0:
                        qt = qk.tile([2 * D, S], DT, tag="qt")
                        kt = qk.tile([2 * D, S], DT, tag="kt")
                        v1t = vp.tile([P, NKC, D1], DT, tag="v1")
                        if not (dma_once and h > 0):
                            nc.sync.dma_start(out=qt, in_=QT[h])
                            nc.sync.dma_start(out=kt, in_=KT[h])
                            nc.sync.dma_start(out=v1t, in_=V1[h])
                        state[h] = {"qt": qt, "kt": kt, "v1": v1t, "pt": {}}
                    qt = state[h]["qt"]
                    kt = state[h]["kt"]

                    while drainq and drainq[0][0] <= idx:
                        drainq.pop(0)[1]()
                    if pv_pos == 0:
                        for _ in range(pv_chunks):
                            if pvq:
                                pop_chunk(idx)
                    elif pv_pos == 2 and pvq:
                        pop_chunk(idx)

                    q0 = P * kc
                    pt = ptp.tile([P, S], DT, tag="pt")
                    first_st = True
                    for a in range(q0, S, st_w):
                        b = min(a + st_w, S)
                        W = b - a
                        st = pst.tile([P, st_w], F32, tag="st")
                        for c, d_ in pieces(0, W, mm_w):
                            base = (mm_i % 2) * D
                            mm_i += 1
                            nc.tensor.matmul(
                                st[:, c:d_],
                                lhsT=kt[base : base + D, kc * P : (kc + 1) * P],
                                rhs=qt[base : base + D, a + c : a + d_],
                                start=True,
                                stop=True,
                            )
                        if not do_exp:
                            continue
                        if pv_pos in (1, 2) and first_st:
                            first_st = False
                            n = pv_chunks if pv_pos == 1 else pv_chunks - 1
                            for _ in range(n):
                                if pvq:
                                    pop_chunk(idx)
                        cost_act = (W + 222) * 0.8333 + 57
                        cost_dve = ((W + 120) * 1.0417 + 70) * dve_scale
                        use_dve = (
                            offload > 0.0
                            and bal["off"] + W <= offload * total_cols
                            and (
                                bal["dve"] + cost_dve < bal["act"] + cost_act
                                or (diag_dve and a == q0)
                            )
                        )
                        if use_dve:
                            nc.vector.tensor_scalar(
                                out=pt[:, a:b].bitcast(I16),
                                in0=st[:, :W],
                                scalar1=float(SCHR_A * SCALE),
                                scalar2=float(schr_b),
                                op0=mybir.AluOpType.mult,
                                op1=mybir.AluOpType.add,
                            )
                            bal["dve"] += cost_dve
                            bal["off"] += W
                        else:
                            nc.scalar.activation(
                                out=pt[:, a:b],
                                in_=st[:, :W],
                                func=EXP,
                                scale=SCALE,
                            )
                            bal["act"] += cost_act
                    if tri_eng == "gpsimd":
                        nc.gpsimd.tensor_mul(pt[:, q0 : q0 + P], pt[:, q0 : q0 + P], tri_t)
                    else:
                        nc.vector.tensor_mul(pt[:, q0 : q0 + P], pt[:, q0 : q0 + P], tri_t)
                        bal["dve"] += (P + 120) * 1.0417 * 0.5 + 100
                    state[h]["pt"][kc] = pt
                    if kc == SH // P - 1:
                        enq_half(h, 0)
                    elif kc == NKC - 1:
                        enq_half(h, 1)

                while pvq:
                    pop_chunk(HPC * NKC)
                while drainq:
                    drainq.pop(0)[1]()

    nc.finalize()
    return nc


# ---------------------------------------------------------------------------
# Fallback builder (general mask / no mask), from the baseline implementation.
# ---------------------------------------------------------------------------


def build_nc(
    causal: bool,
    has_mask: bool = True,
    mm: str = "f32r",
    reps: int = 1,
    st_w: int = 1024,
    pv_lag: int = 1,
    pt_bufs: int = 6,
    qk_bufs: int = 3,
    loop_reps: int = 0,
):
    DT = MM_DTYPES[mm]
    nc = bacc.Bacc(None, target_bir_lowering=False)

    QT = nc.declare_dram_parameter("QT", [HPC, 2 * D, S], DT, isOutput=False)
    KT = nc.declare_dram_parameter("KT", [HPC, 2 * D, S], DT, isOutput=False)
    V1 = nc.declare_dram_parameter("V1", [HPC, S, D1], DT, isOutput=False)
    if causal:
        TRI = nc.declare_dram_parameter("TRI", [P, P], DT, isOutput=False)
    elif has_mask:
        MSKT = nc.declare_dram_parameter("MSKT", [S, S], DT, isOutput=False)
    OT = nc.declare_dram_parameter("OT", [HPC, 2 * D, S // 2], F32, isOutput=True)
    DEN = nc.declare_dram_parameter("DEN", [HPC, 1, S], F32, isOutput=True)

    with tile.TileContext(nc) as tc:
        with (
            tc.tile_pool(name="const", bufs=1) as const,
            tc.tile_pool(name="qk", bufs=qk_bufs) as qk,
            tc.tile_pool(name="vp", bufs=qk_bufs) as vp,
            tc.tile_pool(name="ptp", bufs=pt_bufs) as ptp,
            tc.tile_pool(name="outp", bufs=2) as outp,
            tc.tile_pool(name="mpool", bufs=3) as mpool,
            tc.tile_pool(name="pst", bufs=2, space="PSUM") as pst,
            tc.tile_pool(name="pso", bufs=1, space="PSUM") as pso,
        ):
            if causal:
                tri_t = const.tile([P, P], DT)
                nc.sync.dma_start(out=tri_t, in_=TRI[:])

            import contextlib

            loop_ctx = (
                tc.For_i(0, loop_reps, 1) if loop_reps else contextlib.nullcontext()
            )
            with loop_ctx:
                for h in range(HPC * reps):
                    h = h % HPC
                    qt = qk.tile([2 * D, S], DT, tag="qt")
                    kt = qk.tile([2 * D, S], DT, tag="kt")
                    v1 = vp.tile([P, NKC, D1], DT, tag="v1")
                    nc.sync.dma_start(out=qt, in_=QT[h])
                    nc.sync.dma_start(out=kt, in_=KT[h])
                    nc.sync.dma_start(
                        out=v1, in_=V1[h].rearrange("(c p) d -> p c d", p=P)
                    )

                    ot_ps = pso.tile([D1, S], F32, tag="ot")
                    pending = []

                    def emit_pv(kc, pt, q0):
                        for a, b in pieces(q0, S):
                            last_kc = min(NKC - 1, (b - 1) // P) if causal else NKC - 1
                            nc.tensor.matmul(
                                ot_ps[:, a:b],
                                lhsT=v1[:, kc, :],
                                rhs=pt[:, a:b],
                                start=(kc == 0),
                                stop=(kc == last_kc),
                            )

                    mm_i = 0  # alternates the PE row group per matmul
                    for kc in range(NKC):
                        if len(pending) > pv_lag:
                            emit_pv(*pending.pop(0))
                        q0 = P * kc if causal else 0
                        pt = ptp.tile([P, S], DT, tag="pt")
                        for a in range(q0, S, st_w):
                            b = min(a + st_w, S)
                            st = pst.tile([P, st_w], F32, tag="st")
                            for c, d_ in pieces(0, b - a):
                                base = (mm_i % 2) * D
                                lhsT = kt[base : base + D, kc * P : (kc + 1) * P]
                                mm_i += 1
                                nc.tensor.matmul(
                                    st[:, c:d_],
                                    lhsT=lhsT,
                                    rhs=qt[base : base + D, a + c : a + d_],
                                    start=True,
                                    stop=True,
                                )
                            nc.scalar.activation(
                                out=pt[:, a:b],
                                in_=st[:, : b - a],
                                func=EXP,
                                scale=SCALE,
                            )
                        if causal:
                            nc.vector.tensor_mul(
                                pt[:, q0 : q0 + P], pt[:, q0 : q0 + P], tri_t
                            )
                        elif has_mask:
                            mt = mpool.tile([P, S], DT, tag="mt")
                            nc.sync.dma_start(
                                out=mt, in_=MSKT[kc * P : (kc + 1) * P, :]
                            )
                            nc.vector.tensor_mul(pt, pt, mt)
                        pending.append((kc, pt, q0))
                    for args in pending:
                        emit_pv(*args)
                    ot_sb = outp.tile([2 * D, S // 2], F32, tag="ot_sb")
                    den_sb = outp.tile([1, S], F32, tag="den_sb")
                    nc.vector.tensor_copy(ot_sb[:D, :], ot_ps[:D, : S // 2])
                    nc.vector.tensor_copy(ot_sb[D:, :], ot_ps[:D, S // 2 :])
                    nc.vector.tensor_copy(den_sb, ot_ps[D:D1, :])
                    nc.scalar.dma_start(out=OT[h], in_=ot_sb)
                    nc.scalar.dma_start(out=DEN[h], in_=den_sb)

    nc.finalize()
    return nc


_CACHE = {}


def _get_nc2(mm="f16", **kw):
    key = ("v2", mm, tuple(sorted(kw.items())))
    if key not in _CACHE:
        _CACHE[key] = build_nc2(mm, **kw)
    return _CACHE[key]


def _get_nc(causal, has_mask, mm="f32r", reps=1, **kw):
    key = (causal, has_mask, mm, reps, tuple(sorted(kw.items())))
    if key not in _CACHE:
        _CACHE[key] = build_nc(causal, has_mask, mm, reps, **kw)
    return _CACHE[key]


def _prep_inputs(Q, K, V, mask, mm="f16"):
    """Host-side shard + layout prep. Returns (in_maps, causal, has_mask)."""
    npdt = MM_NP[mm]
    Q = np.ascontiguousarray(np.asarray(Q, dtype=np.float32)).reshape(HEADS, S, D)
    K = np.ascontiguousarray(np.asarray(K, dtype=np.float32)).reshape(HEADS, S, D)
    V = np.ascontiguousarray(np.asarray(V, dtype=np.float32)).reshape(HEADS, S, D)

    has_mask = mask is not None
    causal = False
    if has_mask:
        mask = np.asarray(mask)
        assert mask.shape == (S, S)
        mb = mask.astype(bool)
        causal = bool(np.array_equal(mb, np.tril(np.ones((S, S), dtype=bool))))

    tri = None
    mskt = None
    if causal:
        tri = np.tril(np.ones((P, P), dtype=npdt)).T.copy()
        # keep iff q_local >= k_local
    elif has_mask:
        mskt = np.ascontiguousarray(mb.T).astype(npdt)

    ones = np.ones((HPC, S, 1), dtype=np.float32)
    in_maps = []
    for c in range(NCORES):
        sl = slice(c * HPC, (c + 1) * HPC)
        # [HPC, D, S] transposed then duplicated on the partition axis
        qt_ = Q[sl].transpose(0, 2, 1)
        kt_ = K[sl].transpose(0, 2, 1)
        qs = np.ascontiguousarray(np.concatenate([qt_, qt_], axis=1)).astype(npdt)
        ks = np.ascontiguousarray(np.concatenate([kt_, kt_], axis=1)).astype(npdt)
        # [HPC, S, D+1] -> [HPC, P, NKC, D1] so the device DMA is contiguous
        v1 = np.concatenate([V[sl], ones], axis=2).reshape(HPC, NKC, P, D1)
        v1 = np.ascontiguousarray(v1.transpose(0, 2, 1, 3)).astype(npdt)
        m = {"QT": qs, "KT": ks, "V1": v1}
        if causal:
            m["TRI"] = tri
        elif has_mask:
            m["MSKT"] = mskt
        in_maps.append(m)
    return in_maps, causal, has_mask


def _postprocess2(results):
    """Per-core OTD [HPC, D1, S] fp16 -> full output [B, H, S, D] fp32."""
    outs = []
    for c in range(NCORES):
        otd = results[c]["OTD"].astype(np.float32)
        o = otd[:, :D, :] / otd[:, D : D + 1, :]
        outs.append(o.transpose(0, 2, 1))
    full = np.concatenate(outs, axis=0).reshape(B, H, S, D)
    return np.ascontiguousarray(full.astype(np.float32))


def _postprocess(results):
    """Per-core OT [HPC, 2D, S/2]+DEN -> full output [B, H, S, D] (fallback)."""
    outs = []
    for c in range(NCORES):
        ot = results[c]["OT"].reshape(HPC, 2, D, S // 2)
        den = results[c]["DEN"].reshape(HPC, 2, 1, S // 2)
        o = ot / den
        outs.append(o.transpose(0, 1, 3, 2).reshape(HPC, S, D))
    full = np.concatenate(outs, axis=0).reshape(B, H, S, D)
    return np.ascontiguousarray(full.astype(np.float32))


def run(Q, K, V, mask, trace=False, mm="f16", v2_kwargs=None, **spmd_kwargs):
    in_maps, causal, has_mask = _prep_inputs(Q, K, V, mask, mm)
    if causal:
        nc = _get_nc2(mm, **(v2_kwargs or {}))
        # v2 uses the rearranged V1 layout; fallback maps share prep
        res = run_bass_kernel_spmd(
            nc, in_maps, list(range(NCORES)), trace=trace, **spmd_kwargs
        )
        return _postprocess2(res.results), res
    # fallback path expects V1 as [HPC, S, D1]
    for m in in_maps:
        m["V1"] = np.ascontiguousarray(
            m["V1"].transpose(0, 2, 1, 3).reshape(HPC, S, D1)
        )
    nc = _get_nc(causal, has_mask, mm)
    res = run_bass_kernel_spmd(
        nc, in_maps, list(range(NCORES)), trace=trace, **spmd_kwargs
    )
    return _postprocess(res.results), res


def kernel(Q, K, V, mask=None, **_):
    try:
        out, _res = run(Q, K, V, mask, mm="f16")
    except Exception:
        # transient NRT device-unrecoverable states have been observed to
        # clear on a retry
        import time as _time

        _time.sleep(2.0)
        out, _res = run(Q, K, V, mask, mm="f16")
    return out
